# revision 2
# baseline (speedup 1.0000x reference)
"""Causal self-attention (B=4, S=2048, D=1024, H=16) on 8 trn2 cores.

Sharding: core c -> (batch b = c//2, head-half hh = c%2). Each core:
  - computes Q/K/V projections for its batch restricted to its 8 heads
    (512 of the 1024 feature columns),
  - runs causal attention for those heads,
  - computes a partial out-projection part = attnO @ w_o[rows of its heads].
Host: out[b] = part[2b] + part[2b+1] + (b_v @ w_o + b_o); x is shipped
pre-transposed (feature-major) AND pre-cast to bf16, as are all weights
(the K bias is dropped -- softmax cancels it; the V bias contributes
b_v @ w_o, applied on host).

Key HW facts this version exploits (all measured on the device):
  - matmuls with a 64-row (half-array) stationary run at 2 cycles/row;
    128-row stationaries run at 1 cycle/row.  Scores therefore use a
    128-contract stationary = the head-PAIR's K tile, with the moving Q
    zero-padded on the other head's 64 partitions (exact math).
  - back-to-back matmuls with a rotating 128x128 stationary and a short
    (65-row) moving stream run at ~25ns/instruction (ldweights fully
    pipelined).  PV is therefore FLIPPED: stationary = exp(scores)
    128x128 block, moving = v_aug [128 keys, 65] (64 V columns + ones
    column), accumulating over key tiles into PSUM [128 queries, 65].
    This halves PV moving-rows AND lands the softmax denominator on the
    query PARTITION (cheap per-partition-scalar normalization on
    DVE/Pool instead of a PE rank-1 broadcast).
  - attnO comes out query-major; PE transposes ([128,128] bf16 via
    identity) restore the feature-major layout the out-projection needs.

On-core layouts:
  xt    8 x [128,2048]  bf16 feature-major x (DMA'd directly)
  qz1/2 [128,2048] bf16 Q of head h on its 64 partitions, other 64 ZERO
  kt    [128,2048] bf16 K of both heads of the pair (contract dim)
  v_aug 16 x [128,520]  bf16 token-major, 65 cols/head (64 V + ones)
  st    [128,1024] f32  scores per key tile (2 heads x 512 queries)
  ex    [128,1024] bf16 exp(scores/8), causal-masked; ALL key tiles of a
                        chunk stay live (PV consumes them per q-tile)
  o_ps  [128,130]  f32  PV accumulator per (pair, chunk, qtile):
                        [:,0:65] head A, [:,65:130] head B; col 64/129
                        hold the softmax denominators
  ot    [128,512]  bf16 transposed attnO per (chunk, pair)

Scheduling: per chunk, scores(t)+exp(t) stream with PV(qtile m) groups
slotted in as soon as exp(4j+m) lands; projections for the next pair
(and, for the last pair, the previous chunk's out-projection) zip into
the PE stream as filler, weighted by measured per-instruction ns.
"""

import sys

if "/opt/trn_rl_repo" not in sys.path:
    sys.path.insert(0, "/opt/trn_rl_repo")

import numpy as np
import ml_dtypes

import concourse.bass as bass
import concourse.tile as tile
from concourse import bacc, mybir
from concourse.masks import make_identity

N_CORES = 8
S = 2048
D = 1024
DH = 512          # per-core feature width (8 heads x 64)
HD = 64           # head dim
NH_LOC = 8        # heads per core
F32 = mybir.dt.float32
BF16D = mybir.dt.bfloat16
EXP = mybir.ActivationFunctionType.Exp
GE = mybir.AluOpType.is_ge
NPBF16 = np.dtype(ml_dtypes.bfloat16)

_PROGRAM = None


def _build_program(n_repeat=1):
    nc = bacc.Bacc("TRN2", target_bir_lowering=False, debug=False,
                   num_devices=N_CORES)
    BF16 = mybir.dt.bfloat16
    x_d = nc.dram_tensor("x", [D, S], BF16, kind="ExternalInput").ap()
    wq_d = nc.dram_tensor("wq", [D, DH], BF16, kind="ExternalInput").ap()
    wk_d = nc.dram_tensor("wk", [D, DH], BF16, kind="ExternalInput").ap()
    wv_d = nc.dram_tensor("wv", [D, DH], BF16, kind="ExternalInput").ap()
    wo_d = nc.dram_tensor("wo", [DH, D], BF16, kind="ExternalInput").ap()
    bq_d = nc.dram_tensor("bq", [DH], F32, kind="ExternalInput").ap()
    part_d = nc.dram_tensor("part", [S, D], F32, kind="ExternalOutput").ap()

    with tile.TileContext(nc) as tc:
        for _ in range(n_repeat):
            _emit(nc, tc, x_d, wq_d, wk_d, wv_d, wo_d, bq_d, part_d)
    nc.compile()
    return nc


def _emit(nc, tc, x_d, wq_d, wk_d, wv_d, wo_d, bq_d, part_d):
    from contextlib import ExitStack

    BF16 = mybir.dt.bfloat16
    es = ExitStack()
    with es:
        const_pool = es.enter_context(tc.tile_pool(name="const", bufs=1))
        v_pool = es.enter_context(tc.tile_pool(name="vaug", bufs=1))
        qkv_pool = es.enter_context(tc.tile_pool(name="qkv", bufs=2))
        exp_pool = es.enter_context(tc.tile_pool(name="ex", bufs=18))
        aos_pool = es.enter_context(tc.tile_pool(name="aos", bufs=4))
        aon_pool = es.enter_context(tc.tile_pool(name="aon", bufs=4))

        ones_bf = const_pool.tile([128, NH_LOC], BF16, name="ones_bf",
                                  tag="ones")
        nc.vector.memset(ones_bf, 1.0)
        # Warm the ACT exp table during phase A.
        warm_src = const_pool.tile([1, 1], F32, name="warm_s", tag="warms")
        nc.vector.memset(warm_src, 1.0)
        warm = const_pool.tile([1, 1], F32, name="warm", tag="warm")
        nc.scalar.activation(warm[:], warm_src[:], EXP, scale=1.0)
        ident = const_pool.tile([128, 128], BF16, name="ident", tag="ident")
        make_identity(nc, ident)
        bq_sb = const_pool.tile([128, 4], F32, name="bq_sb", tag="bq")

        v_aug = [v_pool.tile([128, NH_LOC * (HD + 1)], BF16,
                             name=f"va{t}", tag=f"va{t}")
                 for t in range(16)]

        xt_cm = tc.tile_pool(name="xtp", bufs=1, side="right")
        xt_pool = xt_cm.__enter__()
        xt = [xt_pool.tile([128, S], BF16, name=f"xt{k}", tag=f"xt{k}")
              for k in range(8)]
        w_cm = tc.tile_pool(name="wp", bufs=1, side="right")
        wq_pool = w_cm.__enter__()
        wq_t = []
        wk_t = []

        qz = {}      # (g, hl) -> zero-padded Q tile
        kt = {}      # g -> K pair tile
        ot_map = {}

        def emit_proj_units(g, pool):
            """Q/K projection for pair g as (closure, pe_ns) units.

            Q lands in two zero-padded tiles qz[(g,0)]/qz[(g,1)] (the
            other head's 64 partitions stay zero so the score matmul can
            use the full 128-row stationary at 1 cycle/row); K lands in
            kt[g] (both heads on the partition dim, as the scores
            stationary). K gets no bias (softmax cancels it).
            """
            units = []
            state = {}

            def alloc_out(which):
                if which == "kt":
                    t = qkv_pool.tile([128, S], BF16, name=f"kt{g}",
                                      tag="kt")
                    kt[g] = t
                    return t
                t0 = qkv_pool.tile([128, S], BF16, name=f"qz0_{g}",
                                   tag="qz0")
                t1 = qkv_pool.tile([128, S], BF16, name=f"qz1_{g}",
                                   tag="qz1")
                qz[(g, 0)] = t0
                qz[(g, 1)] = t1
                if g < 2:
                    # zero the dead halves once per SBUF slot; later
                    # pairs reuse the slot and the zeros persist.
                    nc.gpsimd.memset(t0[64:128, :], 0.0)
                    nc.gpsimd.memset(t1[0:64, :], 0.0)
                return (t0, t1)

            def mk_kc(which, mc, kc):
                def u():
                    if which not in state:
                        state[which] = alloc_out(which)
                    wt = wq_t if which == "qt" else wk_t
                    if kc == 0:
                        state["pp"] = pool.tile(
                            [128, 512], F32,
                            name=f"pp{which}{g}_{mc}", tag="pp")
                    nc.tensor.matmul(
                        state["pp"][:],
                        wt[kc][:, g * 128:(g + 1) * 128],
                        xt[kc][:, mc * 512:(mc + 1) * 512],
                        start=(kc == 0), stop=(kc == 7))
                    if kc == 7:
                        pp = state.pop("pp")
                        cs = slice(mc * 512, (mc + 1) * 512)
                        if which == "qt":
                            t0, t1 = state["qt"]
                            nc.vector.tensor_scalar_add(
                                t0[0:64, cs], pp[0:64, :],
                                bq_sb[0:64, g:g + 1])
                            nc.vector.tensor_scalar_add(
                                t1[64:128, cs], pp[64:128, :],
                                bq_sb[64:128, g:g + 1])
                        else:
                            nc.vector.tensor_copy(
                                state["kt"][:, cs], pp[:])
                return u

            for which in ("qt", "kt"):
                for mc in range(4):
                    for kc in range(8):
                        units.append((mk_kc(which, mc, kc), 245))
            return units

        # ---- Phase A: stream xT + wv in, V projection zipped ----------
        with (
            tc.tile_pool(name="wv", bufs=8, side="right") as wv_pool,
            tc.tile_pool(name="ppb", bufs=2, space="PSUM") as ppb_pool,
        ):
            wtv = []
            for kc in range(8):
                w_t = wv_pool.tile([128, DH], BF16, name=f"wv{kc}", tag="wv")
                nc.sync.dma_start(
                    out=w_t, in_=wv_d[kc * 128:(kc + 1) * 128, :])
                wtv.append(w_t)
                nc.sync.dma_start(
                    out=xt[kc][:, 0:128],
                    in_=x_d[kc * 128:(kc + 1) * 128, 0:128])
            for kc in range(8):
                nc.sync.dma_start(
                    out=xt[kc][:, 128:512],
                    in_=x_d[kc * 128:(kc + 1) * 128, 128:512])
            for mg in range(1, 4):
                cs = slice(mg * 512, (mg + 1) * 512)
                for kc in range(8):
                    nc.sync.dma_start(
                        out=xt[kc][:, cs],
                        in_=x_d[kc * 128:(kc + 1) * 128, cs])
            for g in range(4):
                sl = slice(g * 128, (g + 1) * 128)
                nc.sync.dma_start(
                    out=bq_sb[:, g:g + 1],
                    in_=bq_d[sl].rearrange("(p one) -> p one", one=1))
            for mt in range(16):
                pp = ppb_pool.tile([128, 512], F32,
                                   name=f"ppv{mt}", tag="pp")
                for kc in range(8):
                    nc.tensor.matmul(
                        pp[:],
                        xt[kc][:, mt * 128:(mt + 1) * 128],
                        wtv[kc][:],
                        start=(kc == 0), stop=(kc == 7))
                va3 = v_aug[mt].rearrange("p (h c) -> p h c", h=NH_LOC)
                nc.vector.tensor_copy(
                    va3[:, :, 0:HD],
                    pp[:].rearrange("p (h c) -> p h c", h=NH_LOC))
                nc.vector.tensor_copy(
                    va3[:, :, HD:HD + 1],
                    ones_bf[:, 0:NH_LOC].rearrange(
                        "p (h one) -> p h one", one=1))
            for kc in range(8):
                w_t = wq_pool.tile([128, DH], BF16, name=f"wqa{kc}",
                                   tag=f"wq{kc}")
                nc.sync.dma_start(
                    out=w_t, in_=wq_d[kc * 128:(kc + 1) * 128, :])
                wq_t.append(w_t)
            for kc in range(8):
                w_t = wq_pool.tile([128, DH], BF16, name=f"wka{kc}",
                                   tag=f"wk{kc}")
                nc.sync.dma_start(
                    out=w_t, in_=wk_d[kc * 128:(kc + 1) * 128, :])
                wk_t.append(w_t)
            for u, _ in emit_proj_units(0, ppb_pool):
                u()

        # ---- attention-phase PSUM pools (8 banks total) ---------------
        st_pool = es.enter_context(
            tc.tile_pool(name="st", bufs=2, space="PSUM"))     # 4 banks
        pv_pool = es.enter_context(
            tc.tile_pool(name="pvp", bufs=2, space="PSUM"))    # 2 banks
        pp_pool = es.enter_context(
            tc.tile_pool(name="pp", bufs=1, space="PSUM"))     # 1 bank
        tp_pool = es.enter_context(
            tc.tile_pool(name="tp", bufs=1, space="PSUM"))     # 1 bank
        ot_pool = es.enter_context(tc.tile_pool(name="otl", bufs=16))

        ex_map = {}

        def emit_scores_exp(g, j, t):
            # Columns [0, z) of a diagonal tile are fully masked: skip
            # them entirely; the 128-col straddle gets affine_select.
            d = t - 4 * j
            z = 0 if d < 0 else 128 * d
            w = 512 - z
            mq = slice(j * 512 + z, (j + 1) * 512)
            nk = slice(t * 128, (t + 1) * 128)
            st = st_pool.tile([128, 1024], F32,
                              name=f"st{j}_{g}_{t}", tag="st")
            for hl in range(2):
                nc.tensor.matmul(
                    st[:, hl * 512 + z:hl * 512 + 512],
                    kt[g][:, nk], qz[(g, hl)][:, mq],
                    start=True, stop=True)
            ex = exp_pool.tile([128, 1024], BF16,
                               name=f"ex{j}_{g}_{t}", tag="ex")
            if d < 0:
                nc.scalar.activation(ex[:, 0:1024], st[:, 0:1024],
                                     EXP, scale=0.125)
            else:
                st3 = st.rearrange("p (h q) -> p h q", h=2)[:, :, z:512]
                ex3 = ex.rearrange("p (h q) -> p h q", h=2)[:, :, z:512]
                nc.scalar.activation(ex3, st3, EXP, scale=0.125)
                # keep where local_mq >= local_nk on the straddle block
                exb = ex.rearrange("p (h q) -> p h q",
                                   h=2)[:, :, z:z + 128]
                nc.gpsimd.affine_select(
                    out=exb, in_=exb,
                    compare_op=GE, fill=0.0, base=0,
                    channel_multiplier=-1,
                    pattern=[[0, 2], [1, 128]])
            ex_map[(g, j, t)] = ex

        def emit_pv(g, j, m):
            # PV for query tile m of chunk j: accumulate over key tiles
            # t=0..4j+m with stationary ex blocks and the 65-row v_aug
            # moving.  Output [128 queries, 65] per head in one PSUM
            # tile; col 64/129 are the softmax denominators.
            t_max = 4 * j + m
            o_ps = pv_pool.tile([128, 130], F32,
                                name=f"o{j}_{g}_{m}", tag="ops")
            ot_map[("ps", g, j, m)] = o_ps
            for hl in range(2):
                h = 2 * g + hl
                for t in range(t_max + 1):
                    nc.tensor.matmul(
                        o_ps[:, hl * 65:hl * 65 + 65],
                        ex_map[(g, j, t)][:, hl * 512 + m * 128:
                                          hl * 512 + (m + 1) * 128],
                        v_aug[t][:, 65 * h:65 * h + 65],
                        start=(t == 0), stop=(t == t_max),
                        skip_group_check=True)

        def emit_norm(g, j, m):
            # DVE: drain PSUM + invert the denominators; Pool: scale.
            o_ps = ot_map.pop(("ps", g, j, m))
            ao_s = aos_pool.tile([128, 130], F32,
                                 name=f"as{j}_{g}_{m}", tag="aos")
            nc.vector.tensor_copy(ao_s[:], o_ps[:])
            rec = ao_s.rearrange("p (h c) -> p h c", h=2)[:, :, 64:65]
            with nc.allow_low_precision(reason="softmax recip"):
                nc.vector.reciprocal(rec, rec)
            ao_n = aon_pool.tile([128, 128], BF16,
                                 name=f"an{j}_{g}_{m}", tag="aon")
            for hl in range(2):
                nc.gpsimd.tensor_scalar_mul(
                    ao_n[:, hl * 64:hl * 64 + 64],
                    ao_s[:, hl * 65:hl * 65 + 64],
                    ao_s[:, hl * 65 + 64:hl * 65 + 65])
            ot_map[("an", g, j, m)] = ao_n

        def emit_tr(g, j, m):
            # PE transpose back to feature-major for the out-projection.
            ao_n = ot_map.pop(("an", g, j, m))
            if (j, g) not in ot_map:
                ot_map[(j, g)] = ot_pool.tile(
                    [128, 512], BF16, name=f"ot{j}_{g}", tag="ot")
            tp = tp_pool.tile([128, 128], BF16,
                              name=f"tp{j}_{g}_{m}", tag="tp")
            nc.tensor.transpose(tp[:], ao_n[:], ident[:])
            nc.vector.tensor_copy(
                ot_map[(j, g)][:, m * 128:(m + 1) * 128], tp[:])

        def att_unit_groups(g, carry_in):
            """Per-chunk unit lists for pair g; returns (groups, carry).

            Chunk j: sc(0..4j), pv(0), sc(4j+1), pv(1)+norm(0),
            sc(4j+2), pv(2)+norm(1)+tr(0), sc(4j+3), pv(3)+norm(2)+
            tr(1), norm(3)+tr(2); tr(3) carries into the next chunk.
            """
            groups = []
            carry = carry_in

            def mk(f, *a):
                return lambda: f(*a)

            for j in range(4):
                def wsc(t, j=j):
                    d = t - 4 * j
                    w = 512 - (0 if d < 0 else 128 * d)
                    return 2 * (int(0.417 * w) + 30)

                units = []
                for t in range(4 * j + 1):
                    units.append((mk(emit_scores_exp, g, j, t), wsc(t)))
                    if carry is not None:
                        units.append((carry, 90))
                        carry = None
                units.append((mk(emit_pv, g, j, 0), 54 * (4 * j + 1)))
                for d in range(1, 4):
                    units.append((mk(emit_scores_exp, g, j, 4 * j + d),
                                  wsc(4 * j + d)))
                    units.append((mk(emit_pv, g, j, d),
                                  54 * (4 * j + d + 1)))
                    units.append((mk(emit_norm, g, j, d - 1), 0))
                    if d >= 2:
                        units.append((mk(emit_tr, g, j, d - 2), 90))
                units.append((mk(emit_norm, g, j, 3), 0))
                units.append((mk(emit_tr, g, j, 2), 90))
                carry = mk(emit_tr, g, j, 3)
                groups.append(units)
            return groups, carry

        def zip_emit(primary, filler):
            tot_p = sum(c for _, c in primary) or 1
            tot_f = sum(c for _, c in filler) or 1
            n_f = len(filler)
            fi = 0
            cum_p = 0
            cum_f = 0
            for u, c in primary:
                u()
                cum_p += c
                while fi < n_f and cum_f * tot_p <= cum_p * tot_f:
                    fu, fc = filler[fi]
                    fu()
                    cum_f += fc
                    fi += 1
            while fi < n_f:
                filler[fi][0]()
                fi += 1

        carry = None
        for g in range(3):
            groups, carry = att_unit_groups(g, carry)
            zip_emit([u for grp in groups for u in grp],
                     emit_proj_units(g + 1, pp_pool))

        w_cm.__exit__(None, None, None)
        xt_cm.__exit__(None, None, None)

        wo_pool = es.enter_context(tc.tile_pool(name="wo", bufs=4))
        os_pool = es.enter_context(tc.tile_pool(name="os", bufs=4))
        wo_t = []
        for fc in range(4):
            w_t = wo_pool.tile([128, D], BF16, name=f"wo{fc}", tag=f"wo{fc}")
            nc.sync.dma_start(
                out=w_t, in_=wo_d[fc * 128:(fc + 1) * 128, :])
            wo_t.append(w_t)

        def outproj_units(j):
            units = []
            for mt in range(4 * j, 4 * j + 4):
                for nck in range(2):
                    st8 = {}

                    def mk_g(j=j, mt=mt, nck=nck, g=0, st8=st8):
                        def u():
                            msl = slice((mt - 4 * j) * 128,
                                        (mt - 4 * j) * 128 + 128)
                            if g == 0:
                                st8["op"] = pp_pool.tile(
                                    [128, 512], F32,
                                    name=f"op{mt}_{nck}", tag="pp")
                            nc.tensor.matmul(
                                st8["op"][:],
                                ot_map[(j, g)][:, msl],
                                wo_t[g][:, nck * 512:(nck + 1) * 512],
                                start=(g == 0), stop=(g == 3))
                            if g == 3:
                                op = st8.pop("op")
                                osb = os_pool.tile(
                                    [128, 512], F32,
                                    name=f"os{mt}_{nck}", tag="os")
                                nc.vector.tensor_copy(osb[:], op[:])
                                nc.sync.dma_start(
                                    out=part_d[
                                        mt * 128:(mt + 1) * 128,
                                        nck * 512:(nck + 1) * 512],
                                    in_=osb[:])
                        return u

                    for g in range(4):
                        units.append((mk_g(g=g), 245))
            return units

        groups, carry = att_unit_groups(3, carry)
        for j in range(4):
            zip_emit(groups[j], outproj_units(j - 1) if j > 0 else [])
        carry()  # tr(3, 3)
        for u, _ in outproj_units(3):
            u()


def _get_program():
    global _PROGRAM
    if _PROGRAM is None:
        _PROGRAM = _build_program()
    return _PROGRAM


_EXEC = None


def _get_executor():
    global _EXEC
    if _EXEC is None:
        import jax
        from jax.experimental.shard_map import shard_map
        from jax.sharding import Mesh, PartitionSpec

        from concourse import bass2jax

        bass2jax.install_neuronx_cc_hook()
        nc = _get_program()
        part_name = (nc.partition_id_tensor.name
                     if nc.partition_id_tensor else None)
        in_names, out_names, out_avals = [], [], []
        for alloc in nc.m.functions[0].allocations:
            if not isinstance(alloc, mybir.MemoryLocationSet):
                continue
            name = alloc.memorylocations[0].name
            if alloc.kind == "ExternalInput":
                if name != part_name:
                    in_names.append(name)
            elif alloc.kind == "ExternalOutput":
                out_names.append(name)
                out_avals.append(jax.core.ShapedArray(
                    tuple(alloc.tensor_shape), mybir.dt.np(alloc.dtype)))
        n_params = len(in_names)
        all_in = tuple(in_names) + tuple(out_names)
        if part_name is not None:
            all_in = all_in + (part_name,)

        def _body(*args):
            operands = list(args)
            if part_name is not None:
                operands.append(bass2jax.partition_id_tensor())
            outs = bass2jax._bass_exec_p.bind(
                *operands,
                out_avals=tuple(out_avals),
                in_names=all_in,
                out_names=tuple(out_names),
                lowering_input_output_aliases=(),
                sim_require_finite=True,
                sim_require_nnan=True,
                nc=nc)
            return tuple(outs)

        devices = jax.devices()[:N_CORES]
        mesh = Mesh(np.asarray(devices), ("core",))
        n_bufs = n_params + len(out_names)
        mapped = shard_map(_body, mesh=mesh,
                           in_specs=(PartitionSpec("core"),) * n_bufs,
                           out_specs=(PartitionSpec("core"),) * len(out_names),
                           check_rep=False)
        fn = jax.jit(mapped,
                     donate_argnums=tuple(range(n_params, n_bufs)),
                     keep_unused=True)
        fn_nodonate = jax.jit(mapped, keep_unused=True)
        out_shapes = [tuple(a.shape) for a in out_avals]
        _EXEC = (fn, fn_nodonate, in_names, out_names, out_shapes, mesh)
    return _EXEC


def run_cores(in_maps):
    fn, _, in_names, out_names, out_shapes = _get_executor()[:5]
    concat_in = [np.concatenate([in_maps[c][n] for c in range(N_CORES)],
                                axis=0) for n in in_names]
    zeros = [np.zeros((N_CORES * s[0],) + s[1:], np.float32)
             for s in out_shapes]
    outs = fn(*concat_in, *zeros)
    res = []
    for c in range(N_CORES):
        res.append({
            n: np.asarray(outs[i]).reshape((N_CORES,) + out_shapes[i])[c]
            for i, n in enumerate(out_names)})
    return res


def make_in_maps(x, w_q, b_q, w_k, b_k, w_v, b_v, w_o, b_o):
    in_maps = []
    for c in range(N_CORES):
        b, hh = divmod(c, 2)
        cols = slice(hh * DH, (hh + 1) * DH)
        in_maps.append({
            "x": np.ascontiguousarray(x[b].T).astype(NPBF16),
            "wq": np.ascontiguousarray(w_q[:, cols]).astype(NPBF16),
            "wk": np.ascontiguousarray(w_k[:, cols]).astype(NPBF16),
            "wv": np.ascontiguousarray(w_v[:, cols]).astype(NPBF16),
            "wo": np.ascontiguousarray(w_o[cols, :]).astype(NPBF16),
            "bq": np.ascontiguousarray(b_q[cols]),
        })
    return in_maps


def combine(parts, b_v, w_o, b_o):
    corr = (b_v @ w_o + b_o).astype(np.float32)
    out = np.empty((4, S, D), dtype=np.float32)
    for b in range(4):
        out[b] = parts[2 * b] + parts[2 * b + 1] + corr
    return out


def kernel(x, w_q, b_q, w_k, b_k, w_v, b_v, w_o, b_o):
    x = np.asarray(x, dtype=np.float32)
    w_q = np.asarray(w_q, dtype=np.float32)
    b_q = np.asarray(b_q, dtype=np.float32)
    w_k = np.asarray(w_k, dtype=np.float32)
    b_k = np.asarray(b_k, dtype=np.float32)
    w_v = np.asarray(w_v, dtype=np.float32)
    b_v = np.asarray(b_v, dtype=np.float32)
    w_o = np.asarray(w_o, dtype=np.float32)
    b_o = np.asarray(b_o, dtype=np.float32)

    in_maps = make_in_maps(x, w_q, b_q, w_k, b_k, w_v, b_v, w_o, b_o)
    res = run_cores(in_maps)
    parts = [res[c]["part"] for c in range(N_CORES)]
    return combine(parts, b_v, w_o, b_o)


# revision 3
# speedup vs baseline: 1.5010x; 1.5010x over previous
"""Causal self-attention (B=4, S=2048, D=1024, H=16) on 8 trn2 cores.

Sharding: core c -> (batch b = c//2, head-half hh = c%2). Each core:
  - computes Q/K/V projections for its batch restricted to its 8 heads
    (512 of the 1024 feature columns),
  - runs causal attention for those heads,
  - computes a partial out-projection part = attnO @ w_o[rows of its heads].
Host: out[b] = part[2b] + part[2b+1] + (b_v @ w_o + b_o); x is shipped
pre-transposed (feature-major) AND pre-cast to bf16, as are all weights
(the K bias is dropped -- softmax cancels it; the V bias contributes
b_v @ w_o, applied on host).

Key HW facts this version exploits (all measured on the device):
  - matmuls with a 64-row (half-array) stationary run at 2 cycles/row;
    128-row stationaries run at 1 cycle/row.  Scores therefore use a
    128-contract stationary = the head-PAIR's K tile, with the moving Q
    zero-padded on the other head's 64 partitions (exact math).
  - back-to-back matmuls with a rotating 128x128 stationary and a short
    (65-row) moving stream run at ~25ns/instruction (ldweights fully
    pipelined).  PV is therefore FLIPPED: stationary = exp(scores)
    128x128 block, moving = v_aug [128 keys, 65] (64 V columns + ones
    column), accumulating over key tiles into PSUM [128 queries, 65].
    This halves PV moving-rows AND lands the softmax denominator on the
    query PARTITION (cheap per-partition-scalar normalization on
    DVE/Pool instead of a PE rank-1 broadcast).
  - attnO comes out query-major; PE transposes ([128,128] bf16 via
    identity) restore the feature-major layout the out-projection needs.

On-core layouts:
  xt    8 x [128,2048]  bf16 feature-major x (DMA'd directly)
  qz1/2 [128,2048] bf16 Q of head h on its 64 partitions, other 64 ZERO
  kt    [128,2048] bf16 K of both heads of the pair (contract dim)
  v_aug 16 x [128,520]  bf16 token-major, 65 cols/head (64 V + ones)
  st    [128,1024] f32  scores per key tile (2 heads x 512 queries)
  ex    [128,1024] bf16 exp(scores/8), causal-masked; ALL key tiles of a
                        chunk stay live (PV consumes them per q-tile)
  o_ps  [128,130]  f32  PV accumulator per (pair, chunk, qtile):
                        [:,0:65] head A, [:,65:130] head B; col 64/129
                        hold the softmax denominators
  ot    [128,512]  bf16 transposed attnO per (chunk, pair)

Scheduling: per chunk, scores(t)+exp(t) stream with PV(qtile m) groups
slotted in as soon as exp(4j+m) lands; projections for the next pair
(and, for the last pair, the previous chunk's out-projection) zip into
the PE stream as filler, weighted by measured per-instruction ns.
"""

import sys

if "/opt/trn_rl_repo" not in sys.path:
    sys.path.insert(0, "/opt/trn_rl_repo")

import numpy as np
import ml_dtypes

import concourse.bass as bass
import concourse.tile as tile
from concourse import bacc, mybir
from concourse.masks import make_identity

N_CORES = 8
S = 2048
D = 1024
DH = 512          # per-core feature width (8 heads x 64)
HD = 64           # head dim
NH_LOC = 8        # heads per core
F32 = mybir.dt.float32
BF16D = mybir.dt.bfloat16
EXP = mybir.ActivationFunctionType.Exp
GE = mybir.AluOpType.is_ge
NPBF16 = np.dtype(ml_dtypes.bfloat16)

_PROGRAM = None


def _build_program(n_repeat=1):
    nc = bacc.Bacc("TRN2", target_bir_lowering=False, debug=False,
                   num_devices=N_CORES)
    BF16 = mybir.dt.bfloat16
    x_d = nc.dram_tensor("x", [D, S], BF16, kind="ExternalInput").ap()
    wq_d = nc.dram_tensor("wq", [D, DH], BF16, kind="ExternalInput").ap()
    wk_d = nc.dram_tensor("wk", [D, DH], BF16, kind="ExternalInput").ap()
    wv_d = nc.dram_tensor("wv", [D, DH], BF16, kind="ExternalInput").ap()
    wo_d = nc.dram_tensor("wo", [DH, D], BF16, kind="ExternalInput").ap()
    bq_d = nc.dram_tensor("bq", [DH], F32, kind="ExternalInput").ap()
    part_d = nc.dram_tensor("part", [S, D], BF16,
                            kind="ExternalOutput").ap()

    with tile.TileContext(nc) as tc:
        for _ in range(n_repeat):
            _emit(nc, tc, x_d, wq_d, wk_d, wv_d, wo_d, bq_d, part_d)
    nc.compile()
    return nc


def _emit(nc, tc, x_d, wq_d, wk_d, wv_d, wo_d, bq_d, part_d):
    from contextlib import ExitStack

    BF16 = mybir.dt.bfloat16
    es = ExitStack()
    with es:
        const_pool = es.enter_context(tc.tile_pool(name="const", bufs=1))
        v_pool = es.enter_context(tc.tile_pool(name="vaug", bufs=1))
        qkv_pool = es.enter_context(tc.tile_pool(name="qkv", bufs=2))
        exp_pool = es.enter_context(tc.tile_pool(name="ex", bufs=18))
        aos_pool = es.enter_context(tc.tile_pool(name="aos", bufs=4))
        aon_pool = es.enter_context(tc.tile_pool(name="aon", bufs=4))

        ones_bf = const_pool.tile([128, NH_LOC], BF16, name="ones_bf",
                                  tag="ones")
        nc.vector.memset(ones_bf, 1.0)
        # Warm the ACT exp table during phase A.
        warm_src = const_pool.tile([1, 1], F32, name="warm_s", tag="warms")
        nc.vector.memset(warm_src, 1.0)
        warm = const_pool.tile([1, 1], F32, name="warm", tag="warm")
        nc.scalar.activation(warm[:], warm_src[:], EXP, scale=1.0)
        ident = const_pool.tile([128, 128], BF16, name="ident", tag="ident")
        make_identity(nc, ident)
        bq_sb = const_pool.tile([128, 4], F32, name="bq_sb", tag="bq")

        v_aug = [v_pool.tile([128, NH_LOC * (HD + 1)], BF16,
                             name=f"va{t}", tag=f"va{t}")
                 for t in range(16)]

        xt_cm = tc.tile_pool(name="xtp", bufs=1, side="right")
        xt_pool = xt_cm.__enter__()
        xt = [xt_pool.tile([128, S], BF16, name=f"xt{k}", tag=f"xt{k}")
              for k in range(8)]
        w_cm = tc.tile_pool(name="wp", bufs=1, side="right")
        wq_pool = w_cm.__enter__()
        wq_t = []
        wk_t = []

        qz = {}      # (g, hl) -> zero-padded Q tile
        kt = {}      # g -> K pair tile
        ot_map = {}

        def emit_proj_units(g, pool):
            """Q/K projection for pair g as (closure, pe_ns) units.

            Q lands in two zero-padded tiles qz[(g,0)]/qz[(g,1)] (the
            other head's 64 partitions stay zero so the score matmul can
            use the full 128-row stationary at 1 cycle/row); K lands in
            kt[g] (both heads on the partition dim, as the scores
            stationary). K gets no bias (softmax cancels it).
            """
            units = []
            state = {}

            def alloc_out(which):
                if which == "kt":
                    t = qkv_pool.tile([128, S], BF16, name=f"kt{g}",
                                      tag="kt")
                    kt[g] = t
                    return t
                t0 = qkv_pool.tile([128, S], BF16, name=f"qz0_{g}",
                                   tag="qz0")
                t1 = qkv_pool.tile([128, S], BF16, name=f"qz1_{g}",
                                   tag="qz1")
                qz[(g, 0)] = t0
                qz[(g, 1)] = t1
                if g < 2:
                    # zero the dead halves once per SBUF slot; later
                    # pairs reuse the slot and the zeros persist.
                    nc.gpsimd.memset(t0[64:128, :], 0.0)
                    nc.gpsimd.memset(t1[0:64, :], 0.0)
                return (t0, t1)

            def mk_kc(which, mc, kc):
                def u():
                    if which not in state:
                        state[which] = alloc_out(which)
                    wt = wq_t if which == "qt" else wk_t
                    if kc == 0:
                        state["pp"] = pool.tile(
                            [128, 512], F32,
                            name=f"pp{which}{g}_{mc}", tag="pp")
                    nc.tensor.matmul(
                        state["pp"][:],
                        wt[kc][:, g * 128:(g + 1) * 128],
                        xt[kc][:, mc * 512:(mc + 1) * 512],
                        start=(kc == 0), stop=(kc == 7))
                    if kc == 7:
                        pp = state.pop("pp")
                        cs = slice(mc * 512, (mc + 1) * 512)
                        if which == "qt":
                            t0, t1 = state["qt"]
                            nc.vector.tensor_scalar_add(
                                t0[0:64, cs], pp[0:64, :],
                                bq_sb[0:64, g:g + 1])
                            nc.vector.tensor_scalar_add(
                                t1[64:128, cs], pp[64:128, :],
                                bq_sb[64:128, g:g + 1])
                        else:
                            nc.vector.tensor_copy(
                                state["kt"][:, cs], pp[:])
                return u

            for which in ("qt", "kt"):
                for mc in range(4):
                    for kc in range(8):
                        units.append((mk_kc(which, mc, kc), 245))
            return units

        # ---- Phase A: stream xT + wv in, V projection zipped ----------
        with (
            tc.tile_pool(name="wv", bufs=8, side="right") as wv_pool,
            tc.tile_pool(name="ppb", bufs=2, space="PSUM") as ppb_pool,
        ):
            wtv = []
            for kc in range(8):
                w_t = wv_pool.tile([128, DH], BF16, name=f"wv{kc}", tag="wv")
                nc.sync.dma_start(
                    out=w_t, in_=wv_d[kc * 128:(kc + 1) * 128, :])
                wtv.append(w_t)
                nc.sync.dma_start(
                    out=xt[kc][:, 0:128],
                    in_=x_d[kc * 128:(kc + 1) * 128, 0:128])
            for kc in range(8):
                nc.sync.dma_start(
                    out=xt[kc][:, 128:512],
                    in_=x_d[kc * 128:(kc + 1) * 128, 128:512])
            for mg in range(1, 4):
                cs = slice(mg * 512, (mg + 1) * 512)
                for kc in range(8):
                    nc.sync.dma_start(
                        out=xt[kc][:, cs],
                        in_=x_d[kc * 128:(kc + 1) * 128, cs])
            for g in range(4):
                sl = slice(g * 128, (g + 1) * 128)
                nc.sync.dma_start(
                    out=bq_sb[:, g:g + 1],
                    in_=bq_d[sl].rearrange("(p one) -> p one", one=1))
            for mt in range(16):
                pp = ppb_pool.tile([128, 512], F32,
                                   name=f"ppv{mt}", tag="pp")
                for kc in range(8):
                    nc.tensor.matmul(
                        pp[:],
                        xt[kc][:, mt * 128:(mt + 1) * 128],
                        wtv[kc][:],
                        start=(kc == 0), stop=(kc == 7))
                va3 = v_aug[mt].rearrange("p (h c) -> p h c", h=NH_LOC)
                nc.vector.tensor_copy(
                    va3[:, :, 0:HD],
                    pp[:].rearrange("p (h c) -> p h c", h=NH_LOC))
                nc.vector.tensor_copy(
                    va3[:, :, HD:HD + 1],
                    ones_bf[:, 0:NH_LOC].rearrange(
                        "p (h one) -> p h one", one=1))
            for kc in range(8):
                w_t = wq_pool.tile([128, DH], BF16, name=f"wqa{kc}",
                                   tag=f"wq{kc}")
                nc.sync.dma_start(
                    out=w_t, in_=wq_d[kc * 128:(kc + 1) * 128, :])
                wq_t.append(w_t)
            for kc in range(8):
                w_t = wq_pool.tile([128, DH], BF16, name=f"wka{kc}",
                                   tag=f"wk{kc}")
                nc.sync.dma_start(
                    out=w_t, in_=wk_d[kc * 128:(kc + 1) * 128, :])
                wk_t.append(w_t)
            for u, _ in emit_proj_units(0, ppb_pool):
                u()

        # ---- attention-phase PSUM pools (8 banks total); closed after
        # the last transpose so the tail out-projection gets a deep pool
        st_cm = tc.tile_pool(name="st", bufs=2, space="PSUM")   # 4 banks
        st_pool = st_cm.__enter__()
        pv_cm = tc.tile_pool(name="pvp", bufs=2, space="PSUM")  # 2 banks
        pv_pool = pv_cm.__enter__()
        pp_cm = tc.tile_pool(name="pp", bufs=1, space="PSUM")   # 1 bank
        pp_pool = pp_cm.__enter__()
        tp_cm = tc.tile_pool(name="tp", bufs=1, space="PSUM")   # 1 bank
        tp_pool = tp_cm.__enter__()
        ot_pool = es.enter_context(tc.tile_pool(name="otl", bufs=16))

        ex_map = {}

        def emit_scores_exp(g, j, t):
            # Columns [0, z) of a diagonal tile are fully masked: skip
            # them entirely; the 128-col straddle gets affine_select.
            d = t - 4 * j
            z = 0 if d < 0 else 128 * d
            w = 512 - z
            mq = slice(j * 512 + z, (j + 1) * 512)
            nk = slice(t * 128, (t + 1) * 128)
            st = st_pool.tile([128, 1024], F32,
                              name=f"st{j}_{g}_{t}", tag="st")
            for hl in range(2):
                nc.tensor.matmul(
                    st[:, hl * 512 + z:hl * 512 + 512],
                    kt[g][:, nk], qz[(g, hl)][:, mq],
                    start=True, stop=True)
            ex = exp_pool.tile([128, 1024], BF16,
                               name=f"ex{j}_{g}_{t}", tag="ex")
            if d < 0:
                nc.scalar.activation(ex[:, 0:1024], st[:, 0:1024],
                                     EXP, scale=0.125)
            else:
                st3 = st.rearrange("p (h q) -> p h q", h=2)[:, :, z:512]
                ex3 = ex.rearrange("p (h q) -> p h q", h=2)[:, :, z:512]
                nc.scalar.activation(ex3, st3, EXP, scale=0.125)
                # keep where local_mq >= local_nk on the straddle block
                exb = ex.rearrange("p (h q) -> p h q",
                                   h=2)[:, :, z:z + 128]
                nc.gpsimd.affine_select(
                    out=exb, in_=exb,
                    compare_op=GE, fill=0.0, base=0,
                    channel_multiplier=-1,
                    pattern=[[0, 2], [1, 128]])
            ex_map[(g, j, t)] = ex

        def emit_pv(g, j, m):
            # PV for query tile m of chunk j: accumulate over key tiles
            # t=0..4j+m with stationary ex blocks and the 65-row v_aug
            # moving.  Output [128 queries, 65] per head in one PSUM
            # tile; col 64/129 are the softmax denominators.
            t_max = 4 * j + m
            o_ps = pv_pool.tile([128, 130], F32,
                                name=f"o{j}_{g}_{m}", tag="ops")
            ot_map[("ps", g, j, m)] = o_ps
            for hl in range(2):
                h = 2 * g + hl
                for t in range(t_max + 1):
                    nc.tensor.matmul(
                        o_ps[:, hl * 65:hl * 65 + 65],
                        ex_map[(g, j, t)][:, hl * 512 + m * 128:
                                          hl * 512 + (m + 1) * 128],
                        v_aug[t][:, 65 * h:65 * h + 65],
                        start=(t == 0), stop=(t == t_max),
                        skip_group_check=True)

        def emit_norm(g, j, m):
            # DVE: drain PSUM + invert the denominators; Pool: scale.
            o_ps = ot_map.pop(("ps", g, j, m))
            ao_s = aos_pool.tile([128, 130], F32,
                                 name=f"as{j}_{g}_{m}", tag="aos")
            nc.vector.tensor_copy(ao_s[:], o_ps[:])
            rec = ao_s.rearrange("p (h c) -> p h c", h=2)[:, :, 64:65]
            with nc.allow_low_precision(reason="softmax recip"):
                nc.vector.reciprocal(rec, rec)
            ao_n = aon_pool.tile([128, 128], BF16,
                                 name=f"an{j}_{g}_{m}", tag="aon")
            for hl in range(2):
                nc.gpsimd.tensor_scalar_mul(
                    ao_n[:, hl * 64:hl * 64 + 64],
                    ao_s[:, hl * 65:hl * 65 + 64],
                    ao_s[:, hl * 65 + 64:hl * 65 + 65])
            ot_map[("an", g, j, m)] = ao_n

        def emit_tr(g, j, m):
            # PE transpose back to feature-major for the out-projection.
            ao_n = ot_map.pop(("an", g, j, m))
            if (j, g) not in ot_map:
                ot_map[(j, g)] = ot_pool.tile(
                    [128, 512], BF16, name=f"ot{j}_{g}", tag="ot")
            tp = tp_pool.tile([128, 128], BF16,
                              name=f"tp{j}_{g}_{m}", tag="tp")
            nc.tensor.transpose(tp[:], ao_n[:], ident[:])
            nc.vector.tensor_copy(
                ot_map[(j, g)][:, m * 128:(m + 1) * 128], tp[:])

        def att_unit_groups(g, carry_in):
            """Per-chunk unit lists for pair g; returns (groups, carry).

            Chunk j: sc(0..4j), pv(0), sc(4j+1), pv(1)+norm(0),
            sc(4j+2), pv(2)+norm(1)+tr(0), sc(4j+3), pv(3)+norm(2)+
            tr(1), norm(3)+tr(2); tr(3) carries into the next chunk.
            """
            groups = []
            carry = carry_in

            def mk(f, *a):
                return lambda: f(*a)

            for j in range(4):
                def wsc(t, j=j):
                    d = t - 4 * j
                    w = 512 - (0 if d < 0 else 128 * d)
                    return 2 * (int(0.417 * w) + 30)

                units = []
                for t in range(4 * j + 1):
                    units.append((mk(emit_scores_exp, g, j, t), wsc(t)))
                    if carry is not None:
                        units.append((carry, 90))
                        carry = None
                units.append((mk(emit_pv, g, j, 0), 54 * (4 * j + 1)))
                for d in range(1, 4):
                    units.append((mk(emit_scores_exp, g, j, 4 * j + d),
                                  wsc(4 * j + d)))
                    units.append((mk(emit_pv, g, j, d),
                                  54 * (4 * j + d + 1)))
                    units.append((mk(emit_norm, g, j, d - 1), 0))
                    if d >= 2:
                        units.append((mk(emit_tr, g, j, d - 2), 90))
                units.append((mk(emit_norm, g, j, 3), 0))
                units.append((mk(emit_tr, g, j, 2), 90))
                carry = mk(emit_tr, g, j, 3)
                groups.append(units)
            return groups, carry

        def zip_emit(primary, filler):
            tot_p = sum(c for _, c in primary) or 1
            tot_f = sum(c for _, c in filler) or 1
            n_f = len(filler)
            fi = 0
            cum_p = 0
            cum_f = 0
            for u, c in primary:
                u()
                cum_p += c
                while fi < n_f and cum_f * tot_p <= cum_p * tot_f:
                    fu, fc = filler[fi]
                    fu()
                    cum_f += fc
                    fi += 1
            while fi < n_f:
                filler[fi][0]()
                fi += 1

        carry = None
        for g in range(3):
            groups, carry = att_unit_groups(g, carry)
            zip_emit([u for grp in groups for u in grp],
                     emit_proj_units(g + 1, pp_pool))

        w_cm.__exit__(None, None, None)
        xt_cm.__exit__(None, None, None)

        wo_pool = es.enter_context(tc.tile_pool(name="wo", bufs=4))
        os_pool = es.enter_context(tc.tile_pool(name="os", bufs=4))
        wo_t = []
        for fc in range(4):
            w_t = wo_pool.tile([128, D], BF16, name=f"wo{fc}", tag=f"wo{fc}")
            nc.sync.dma_start(
                out=w_t, in_=wo_d[fc * 128:(fc + 1) * 128, :])
            wo_t.append(w_t)

        def outproj_units(j, pool=None):
            units = []
            for mt in range(4 * j, 4 * j + 4):
                for nck in range(2):
                    st8 = {}

                    def mk_g(j=j, mt=mt, nck=nck, g=0, st8=st8):
                        def u():
                            msl = slice((mt - 4 * j) * 128,
                                        (mt - 4 * j) * 128 + 128)
                            if g == 0:
                                st8["op"] = (pool or pp_pool).tile(
                                    [128, 512], F32,
                                    name=f"op{mt}_{nck}", tag="pp")
                            nc.tensor.matmul(
                                st8["op"][:],
                                ot_map[(j, g)][:, msl],
                                wo_t[g][:, nck * 512:(nck + 1) * 512],
                                start=(g == 0), stop=(g == 3))
                            if g == 3:
                                op = st8.pop("op")
                                osb = os_pool.tile(
                                    [128, 512], BF16,
                                    name=f"os{mt}_{nck}", tag="os")
                                nc.vector.tensor_copy(osb[:], op[:])
                                nc.sync.dma_start(
                                    out=part_d[
                                        mt * 128:(mt + 1) * 128,
                                        nck * 512:(nck + 1) * 512],
                                    in_=osb[:])
                        return u

                    for g in range(4):
                        units.append((mk_g(g=g), 245))
            return units

        groups, carry = att_unit_groups(3, carry)
        for j in range(4):
            zip_emit(groups[j], outproj_units(j - 1) if j > 0 else [])
        carry()  # tr(3, 3)
        # Free the attention PSUM pools (LIFO) and give the tail
        # out-projection a deep chain pool so its 8 accumulation chains
        # pipeline instead of serializing on one bank.
        tp_cm.__exit__(None, None, None)
        pp_cm.__exit__(None, None, None)
        pv_cm.__exit__(None, None, None)
        st_cm.__exit__(None, None, None)
        with tc.tile_pool(name="opt", bufs=6, space="PSUM") as op_pool:
            for u, _ in outproj_units(3, pool=op_pool):
                u()


def _get_program():
    global _PROGRAM
    if _PROGRAM is None:
        _PROGRAM = _build_program()
    return _PROGRAM


_EXEC = None


def _get_executor():
    global _EXEC
    if _EXEC is None:
        import jax
        from jax.experimental.shard_map import shard_map
        from jax.sharding import Mesh, PartitionSpec

        from concourse import bass2jax

        bass2jax.install_neuronx_cc_hook()
        nc = _get_program()
        part_name = (nc.partition_id_tensor.name
                     if nc.partition_id_tensor else None)
        in_names, out_names, out_avals = [], [], []
        for alloc in nc.m.functions[0].allocations:
            if not isinstance(alloc, mybir.MemoryLocationSet):
                continue
            name = alloc.memorylocations[0].name
            if alloc.kind == "ExternalInput":
                if name != part_name:
                    in_names.append(name)
            elif alloc.kind == "ExternalOutput":
                out_names.append(name)
                out_avals.append(jax.core.ShapedArray(
                    tuple(alloc.tensor_shape), mybir.dt.np(alloc.dtype)))
        n_params = len(in_names)
        all_in = tuple(in_names) + tuple(out_names)
        if part_name is not None:
            all_in = all_in + (part_name,)

        def _body(*args):
            operands = list(args)
            if part_name is not None:
                operands.append(bass2jax.partition_id_tensor())
            outs = bass2jax._bass_exec_p.bind(
                *operands,
                out_avals=tuple(out_avals),
                in_names=all_in,
                out_names=tuple(out_names),
                lowering_input_output_aliases=(),
                sim_require_finite=True,
                sim_require_nnan=True,
                nc=nc)
            return tuple(outs)

        devices = jax.devices()[:N_CORES]
        mesh = Mesh(np.asarray(devices), ("core",))
        n_bufs = n_params + len(out_names)
        mapped = shard_map(_body, mesh=mesh,
                           in_specs=(PartitionSpec("core"),) * n_bufs,
                           out_specs=(PartitionSpec("core"),) * len(out_names),
                           check_rep=False)
        fn = jax.jit(mapped,
                     donate_argnums=tuple(range(n_params, n_bufs)),
                     keep_unused=True)
        fn_nodonate = jax.jit(mapped, keep_unused=True)
        out_shapes = [tuple(a.shape) for a in out_avals]
        out_dtypes = [a.dtype for a in out_avals]
        _EXEC = (fn, fn_nodonate, in_names, out_names, out_shapes, mesh,
                 out_dtypes)
    return _EXEC


def run_cores(in_maps):
    ex = _get_executor()
    fn, _, in_names, out_names, out_shapes = ex[:5]
    out_dtypes = ex[6]
    concat_in = [np.concatenate([in_maps[c][n] for c in range(N_CORES)],
                                axis=0) for n in in_names]
    zeros = [np.zeros((N_CORES * s[0],) + s[1:], dt)
             for s, dt in zip(out_shapes, out_dtypes)]
    outs = fn(*concat_in, *zeros)
    res = []
    for c in range(N_CORES):
        res.append({
            n: np.asarray(outs[i]).reshape((N_CORES,) + out_shapes[i])[c]
            for i, n in enumerate(out_names)})
    return res


def make_in_maps(x, w_q, b_q, w_k, b_k, w_v, b_v, w_o, b_o):
    in_maps = []
    for c in range(N_CORES):
        b, hh = divmod(c, 2)
        cols = slice(hh * DH, (hh + 1) * DH)
        in_maps.append({
            "x": np.ascontiguousarray(x[b].T).astype(NPBF16),
            "wq": np.ascontiguousarray(w_q[:, cols]).astype(NPBF16),
            "wk": np.ascontiguousarray(w_k[:, cols]).astype(NPBF16),
            "wv": np.ascontiguousarray(w_v[:, cols]).astype(NPBF16),
            "wo": np.ascontiguousarray(w_o[cols, :]).astype(NPBF16),
            "bq": np.ascontiguousarray(b_q[cols]),
        })
    return in_maps


def combine(parts, b_v, w_o, b_o):
    corr = (b_v @ w_o + b_o).astype(np.float32)
    out = np.empty((4, S, D), dtype=np.float32)
    for b in range(4):
        out[b] = (parts[2 * b].astype(np.float32)
                  + parts[2 * b + 1].astype(np.float32) + corr)
    return out


def kernel(x, w_q, b_q, w_k, b_k, w_v, b_v, w_o, b_o):
    x = np.asarray(x, dtype=np.float32)
    w_q = np.asarray(w_q, dtype=np.float32)
    b_q = np.asarray(b_q, dtype=np.float32)
    w_k = np.asarray(w_k, dtype=np.float32)
    b_k = np.asarray(b_k, dtype=np.float32)
    w_v = np.asarray(w_v, dtype=np.float32)
    b_v = np.asarray(b_v, dtype=np.float32)
    w_o = np.asarray(w_o, dtype=np.float32)
    b_o = np.asarray(b_o, dtype=np.float32)

    in_maps = make_in_maps(x, w_q, b_q, w_k, b_k, w_v, b_v, w_o, b_o)
    res = run_cores(in_maps)
    parts = [res[c]["part"] for c in range(N_CORES)]
    return combine(parts, b_v, w_o, b_o)


# revision 4
# speedup vs baseline: 1.6707x; 1.1130x over previous
"""Causal self-attention (B=4, S=2048, D=1024, H=16) on 8 trn2 cores.

Sharding: core c -> (batch b = c//2, head-half hh = c%2). Each core:
  - computes Q/K/V projections for its batch restricted to its 8 heads
    (512 of the 1024 feature columns),
  - runs causal attention for those heads,
  - computes a partial out-projection part = attnO @ w_o[rows of its heads].
Host: out[b] = part[2b] + part[2b+1] + (b_v @ w_o + b_o); x is shipped
pre-transposed (feature-major) AND pre-cast to bf16, as are all weights
(the K bias is dropped -- softmax cancels it; the V bias contributes
b_v @ w_o, applied on host).

Key HW facts this version exploits (all measured on the device):
  - matmuls with a 64-row (half-array) stationary run at 2 cycles/row;
    128-row stationaries run at 1 cycle/row.  Scores therefore use a
    128-contract stationary = the head-PAIR's K tile, with the moving Q
    zero-padded on the other head's 64 partitions (exact math).
  - back-to-back matmuls with a rotating 128x128 stationary and a short
    (65-row) moving stream run at ~25ns/instruction (ldweights fully
    pipelined).  PV is therefore FLIPPED: stationary = exp(scores)
    128x128 block, moving = v_aug [128 keys, 65] (64 V columns + ones
    column), accumulating over key tiles into PSUM [128 queries, 65].
    This halves PV moving-rows AND lands the softmax denominator on the
    query PARTITION (cheap per-partition-scalar normalization on
    DVE/Pool instead of a PE rank-1 broadcast).
  - attnO comes out query-major; PE transposes ([128,128] bf16 via
    identity) restore the feature-major layout the out-projection needs.

On-core layouts:
  xt    8 x [128,2048]  bf16 feature-major x (DMA'd directly)
  qz1/2 [128,2048] bf16 Q of head h on its 64 partitions, other 64 ZERO
  kt    [128,2048] bf16 K of both heads of the pair (contract dim)
  v_aug 16 x [128,520]  bf16 token-major, 65 cols/head (64 V + ones)
  st    [128,1024] f32  scores per key tile (2 heads x 512 queries)
  ex    [128,1024] bf16 exp(scores/8), causal-masked; ALL key tiles of a
                        chunk stay live (PV consumes them per q-tile)
  o_ps  [128,130]  f32  PV accumulator per (pair, chunk, qtile):
                        [:,0:65] head A, [:,65:130] head B; col 64/129
                        hold the softmax denominators
  ot    [128,512]  bf16 transposed attnO per (chunk, pair)

Scheduling: per chunk, scores(t)+exp(t) stream with PV(qtile m) groups
slotted in as soon as exp(4j+m) lands; projections for the next pair
(and, for the last pair, the previous chunk's out-projection) zip into
the PE stream as filler, weighted by measured per-instruction ns.
"""

import sys

if "/opt/trn_rl_repo" not in sys.path:
    sys.path.insert(0, "/opt/trn_rl_repo")

import numpy as np
import ml_dtypes

import concourse.bass as bass
import concourse.tile as tile
from concourse import bacc, mybir
from concourse.masks import make_identity

N_CORES = 8
S = 2048
D = 1024
DH = 512          # per-core feature width (8 heads x 64)
HD = 64           # head dim
NH_LOC = 8        # heads per core
F32 = mybir.dt.float32
BF16D = mybir.dt.bfloat16
EXP = mybir.ActivationFunctionType.Exp
GE = mybir.AluOpType.is_ge
NPBF16 = np.dtype(ml_dtypes.bfloat16)

_PROGRAM = None


def _build_program(n_repeat=1):
    nc = bacc.Bacc("TRN2", target_bir_lowering=False, debug=False,
                   num_devices=N_CORES)
    BF16 = mybir.dt.bfloat16
    x_d = nc.dram_tensor("x", [D, S], BF16, kind="ExternalInput").ap()
    wq_d = nc.dram_tensor("wq", [D, DH], BF16, kind="ExternalInput").ap()
    wk_d = nc.dram_tensor("wk", [D, DH], BF16, kind="ExternalInput").ap()
    wv_d = nc.dram_tensor("wv", [D, DH], BF16, kind="ExternalInput").ap()
    wo_d = nc.dram_tensor("wo", [DH, D], BF16, kind="ExternalInput").ap()
    bq_d = nc.dram_tensor("bq", [DH], F32, kind="ExternalInput").ap()
    part_d = nc.dram_tensor("part", [S, D], BF16,
                            kind="ExternalOutput").ap()

    with tile.TileContext(nc) as tc:
        for _ in range(n_repeat):
            _emit(nc, tc, x_d, wq_d, wk_d, wv_d, wo_d, bq_d, part_d)
    nc.compile()
    return nc


def _emit(nc, tc, x_d, wq_d, wk_d, wv_d, wo_d, bq_d, part_d):
    from contextlib import ExitStack

    BF16 = mybir.dt.bfloat16
    es = ExitStack()
    with es:
        const_pool = es.enter_context(tc.tile_pool(name="const", bufs=1))
        v_pool = es.enter_context(tc.tile_pool(name="vaug", bufs=1))
        qkv_pool = es.enter_context(tc.tile_pool(name="qkv", bufs=2))
        exp_pool = es.enter_context(tc.tile_pool(name="ex", bufs=18))
        aos_pool = es.enter_context(tc.tile_pool(name="aos", bufs=4))
        aon_pool = es.enter_context(tc.tile_pool(name="aon", bufs=4))

        ones_bf = const_pool.tile([128, NH_LOC], BF16, name="ones_bf",
                                  tag="ones")
        nc.vector.memset(ones_bf, 1.0)
        # Warm the ACT exp table during phase A.
        warm_src = const_pool.tile([1, 1], F32, name="warm_s", tag="warms")
        nc.vector.memset(warm_src, 1.0)
        warm = const_pool.tile([1, 1], F32, name="warm", tag="warm")
        nc.scalar.activation(warm[:], warm_src[:], EXP, scale=1.0)
        ident = const_pool.tile([128, 128], BF16, name="ident", tag="ident")
        make_identity(nc, ident)
        bq_sb = const_pool.tile([128, 4], F32, name="bq_sb", tag="bq")
        # Causal straddle mask, twice side-by-side (one per head-half):
        # tri2[p, c] = 1.0 if (c % 128) >= p else 0.0.  Applied with a
        # DVE multiply -- the GPSIMD affine_select costs ~1us per call
        # on HW and sat on the exp->PV critical path.
        tri2 = const_pool.tile([128, 2, 128], BF16, name="tri2", tag="tri2")
        nc.vector.memset(tri2, 1.0)
        nc.gpsimd.affine_select(
            out=tri2, in_=tri2,
            compare_op=GE, fill=0.0, base=0,
            channel_multiplier=-1,
            pattern=[[0, 2], [1, 128]])

        v_aug = [v_pool.tile([128, NH_LOC * (HD + 1)], BF16,
                             name=f"va{t}", tag=f"va{t}")
                 for t in range(16)]

        xt_cm = tc.tile_pool(name="xtp", bufs=1, side="right")
        xt_pool = xt_cm.__enter__()
        xt = [xt_pool.tile([128, S], BF16, name=f"xt{k}", tag=f"xt{k}")
              for k in range(8)]
        w_cm = tc.tile_pool(name="wp", bufs=1, side="right")
        wq_pool = w_cm.__enter__()
        wq_t = []
        wk_t = []

        qz = {}      # (g, hl) -> zero-padded Q tile
        kt = {}      # g -> K pair tile
        ot_map = {}

        def emit_proj_units(g, pool):
            """Q/K projection for pair g as (closure, pe_ns) units.

            Q lands in two zero-padded tiles qz[(g,0)]/qz[(g,1)] (the
            other head's 64 partitions stay zero so the score matmul can
            use the full 128-row stationary at 1 cycle/row); K lands in
            kt[g] (both heads on the partition dim, as the scores
            stationary). K gets no bias (softmax cancels it).
            """
            units = []
            state = {}

            def alloc_out(which):
                if which == "kt":
                    t = qkv_pool.tile([128, S], BF16, name=f"kt{g}",
                                      tag="kt")
                    kt[g] = t
                    return t
                t0 = qkv_pool.tile([128, S], BF16, name=f"qz0_{g}",
                                   tag="qz0")
                t1 = qkv_pool.tile([128, S], BF16, name=f"qz1_{g}",
                                   tag="qz1")
                qz[(g, 0)] = t0
                qz[(g, 1)] = t1
                if g < 2:
                    # zero the dead halves once per SBUF slot; later
                    # pairs reuse the slot and the zeros persist.
                    nc.gpsimd.memset(t0[64:128, :], 0.0)
                    nc.gpsimd.memset(t1[0:64, :], 0.0)
                return (t0, t1)

            def mk_kc(which, mc, kc):
                def u():
                    if which not in state:
                        state[which] = alloc_out(which)
                    wt = wq_t if which == "qt" else wk_t
                    if kc == 0:
                        state["pp"] = pool.tile(
                            [128, 512], F32,
                            name=f"pp{which}{g}_{mc}", tag="pp")
                    nc.tensor.matmul(
                        state["pp"][:],
                        wt[kc][:, g * 128:(g + 1) * 128],
                        xt[kc][:, mc * 512:(mc + 1) * 512],
                        start=(kc == 0), stop=(kc == 7))
                    if kc == 7:
                        pp = state.pop("pp")
                        cs = slice(mc * 512, (mc + 1) * 512)
                        if which == "qt":
                            t0, t1 = state["qt"]
                            nc.vector.tensor_scalar_add(
                                t0[0:64, cs], pp[0:64, :],
                                bq_sb[0:64, g:g + 1])
                            nc.vector.tensor_scalar_add(
                                t1[64:128, cs], pp[64:128, :],
                                bq_sb[64:128, g:g + 1])
                        else:
                            nc.vector.tensor_copy(
                                state["kt"][:, cs], pp[:])
                return u

            for which in ("qt", "kt"):
                for mc in range(4):
                    for kc in range(8):
                        units.append((mk_kc(which, mc, kc), 245))
            return units

        # ---- Phase A: stream xT + wv in, V projection zipped ----------
        with (
            tc.tile_pool(name="wv", bufs=8, side="right") as wv_pool,
            tc.tile_pool(name="ppb", bufs=2, space="PSUM") as ppb_pool,
        ):
            wtv = []
            for kc in range(8):
                w_t = wv_pool.tile([128, DH], BF16, name=f"wv{kc}", tag="wv")
                nc.sync.dma_start(
                    out=w_t, in_=wv_d[kc * 128:(kc + 1) * 128, :])
                wtv.append(w_t)
                nc.sync.dma_start(
                    out=xt[kc][:, 0:128],
                    in_=x_d[kc * 128:(kc + 1) * 128, 0:128])
            for kc in range(8):
                nc.sync.dma_start(
                    out=xt[kc][:, 128:512],
                    in_=x_d[kc * 128:(kc + 1) * 128, 128:512])
            for mg in range(1, 4):
                cs = slice(mg * 512, (mg + 1) * 512)
                for kc in range(8):
                    nc.sync.dma_start(
                        out=xt[kc][:, cs],
                        in_=x_d[kc * 128:(kc + 1) * 128, cs])
            for g in range(4):
                sl = slice(g * 128, (g + 1) * 128)
                nc.sync.dma_start(
                    out=bq_sb[:, g:g + 1],
                    in_=bq_d[sl].rearrange("(p one) -> p one", one=1))
            for mt in range(16):
                pp = ppb_pool.tile([128, 512], F32,
                                   name=f"ppv{mt}", tag="pp")
                for kc in range(8):
                    nc.tensor.matmul(
                        pp[:],
                        xt[kc][:, mt * 128:(mt + 1) * 128],
                        wtv[kc][:],
                        start=(kc == 0), stop=(kc == 7))
                va3 = v_aug[mt].rearrange("p (h c) -> p h c", h=NH_LOC)
                nc.vector.tensor_copy(
                    va3[:, :, 0:HD],
                    pp[:].rearrange("p (h c) -> p h c", h=NH_LOC))
                nc.vector.tensor_copy(
                    va3[:, :, HD:HD + 1],
                    ones_bf[:, 0:NH_LOC].rearrange(
                        "p (h one) -> p h one", one=1))
            for kc in range(8):
                w_t = wq_pool.tile([128, DH], BF16, name=f"wqa{kc}",
                                   tag=f"wq{kc}")
                nc.sync.dma_start(
                    out=w_t, in_=wq_d[kc * 128:(kc + 1) * 128, :])
                wq_t.append(w_t)
            for kc in range(8):
                w_t = wq_pool.tile([128, DH], BF16, name=f"wka{kc}",
                                   tag=f"wk{kc}")
                nc.sync.dma_start(
                    out=w_t, in_=wk_d[kc * 128:(kc + 1) * 128, :])
                wk_t.append(w_t)
            for u, _ in emit_proj_units(0, ppb_pool):
                u()

        # ---- attention-phase PSUM pools (8 banks total); closed after
        # the last transpose so the tail out-projection gets a deep pool
        st_cm = tc.tile_pool(name="st", bufs=2, space="PSUM")   # 4 banks
        st_pool = st_cm.__enter__()
        pv_cm = tc.tile_pool(name="pvp", bufs=2, space="PSUM")  # 2 banks
        pv_pool = pv_cm.__enter__()
        pp_cm = tc.tile_pool(name="pp", bufs=1, space="PSUM")   # 1 bank
        pp_pool = pp_cm.__enter__()
        tp_cm = tc.tile_pool(name="tp", bufs=1, space="PSUM")   # 1 bank
        tp_pool = tp_cm.__enter__()
        ot_pool = es.enter_context(tc.tile_pool(name="otl", bufs=16))

        ex_map = {}

        def emit_scores_exp(g, j, t):
            # Columns [0, z) of a diagonal tile are fully masked: skip
            # them entirely; the 128-col straddle gets affine_select.
            d = t - 4 * j
            z = 0 if d < 0 else 128 * d
            w = 512 - z
            mq = slice(j * 512 + z, (j + 1) * 512)
            nk = slice(t * 128, (t + 1) * 128)
            st = st_pool.tile([128, 1024], F32,
                              name=f"st{j}_{g}_{t}", tag="st")
            for hl in range(2):
                nc.tensor.matmul(
                    st[:, hl * 512 + z:hl * 512 + 512],
                    kt[g][:, nk], qz[(g, hl)][:, mq],
                    start=True, stop=True)
            ex = exp_pool.tile([128, 1024], BF16,
                               name=f"ex{j}_{g}_{t}", tag="ex")
            if d < 0:
                nc.scalar.activation(ex[:, 0:1024], st[:, 0:1024],
                                     EXP, scale=0.125)
            else:
                st3 = st.rearrange("p (h q) -> p h q", h=2)[:, :, z:512]
                ex3 = ex.rearrange("p (h q) -> p h q", h=2)[:, :, z:512]
                nc.scalar.activation(ex3, st3, EXP, scale=0.125)
                # zero above the diagonal on the straddle block (DVE)
                exb = ex.rearrange("p (h q) -> p h q",
                                   h=2)[:, :, z:z + 128]
                nc.vector.tensor_mul(exb, exb, tri2[:])
            ex_map[(g, j, t)] = ex

        def emit_pv(g, j, m):
            # PV for query tile m of chunk j: accumulate over key tiles
            # t=0..4j+m with stationary ex blocks and the 65-row v_aug
            # moving.  Output [128 queries, 65] per head in one PSUM
            # tile; col 64/129 are the softmax denominators.
            t_max = 4 * j + m
            o_ps = pv_pool.tile([128, 130], F32,
                                name=f"o{j}_{g}_{m}", tag="ops")
            ot_map[("ps", g, j, m)] = o_ps
            for hl in range(2):
                h = 2 * g + hl
                for t in range(t_max + 1):
                    nc.tensor.matmul(
                        o_ps[:, hl * 65:hl * 65 + 65],
                        ex_map[(g, j, t)][:, hl * 512 + m * 128:
                                          hl * 512 + (m + 1) * 128],
                        v_aug[t][:, 65 * h:65 * h + 65],
                        start=(t == 0), stop=(t == t_max),
                        skip_group_check=True)

        def emit_norm(g, j, m):
            # DVE: drain PSUM + invert the denominators; Pool: scale.
            o_ps = ot_map.pop(("ps", g, j, m))
            ao_s = aos_pool.tile([128, 130], F32,
                                 name=f"as{j}_{g}_{m}", tag="aos")
            nc.vector.tensor_copy(ao_s[:], o_ps[:])
            rec = ao_s.rearrange("p (h c) -> p h c", h=2)[:, :, 64:65]
            with nc.allow_low_precision(reason="softmax recip"):
                nc.vector.reciprocal(rec, rec)
            ao_n = aon_pool.tile([128, 128], BF16,
                                 name=f"an{j}_{g}_{m}", tag="aon")
            for hl in range(2):
                nc.vector.tensor_scalar_mul(
                    ao_n[:, hl * 64:hl * 64 + 64],
                    ao_s[:, hl * 65:hl * 65 + 64],
                    ao_s[:, hl * 65 + 64:hl * 65 + 65])
            ot_map[("an", g, j, m)] = ao_n

        def emit_tr(g, j, m):
            # PE transpose back to feature-major for the out-projection.
            ao_n = ot_map.pop(("an", g, j, m))
            if (j, g) not in ot_map:
                ot_map[(j, g)] = ot_pool.tile(
                    [128, 512], BF16, name=f"ot{j}_{g}", tag="ot")
            tp = tp_pool.tile([128, 128], BF16,
                              name=f"tp{j}_{g}_{m}", tag="tp")
            nc.tensor.transpose(tp[:], ao_n[:], ident[:])
            nc.vector.tensor_copy(
                ot_map[(j, g)][:, m * 128:(m + 1) * 128], tp[:])

        def att_unit_groups(g, carry_in):
            """Per-chunk unit lists for pair g; returns (groups, carry).

            Chunk j: sc(0..4j), pv(0), sc(4j+1), pv(1)+norm(0),
            sc(4j+2), pv(2)+norm(1)+tr(0), sc(4j+3), pv(3)+norm(2)+
            tr(1), norm(3)+tr(2); tr(3) carries into the next chunk.
            """
            groups = []
            carry = carry_in

            def mk(f, *a):
                return lambda: f(*a)

            for j in range(4):
                def wsc(t, j=j):
                    d = t - 4 * j
                    w = 512 - (0 if d < 0 else 128 * d)
                    return 2 * (int(0.417 * w) + 30)

                units = []
                for t in range(4 * j + 1):
                    units.append((mk(emit_scores_exp, g, j, t), wsc(t)))
                    if carry is not None:
                        units.append((carry, 90))
                        carry = None
                units.append((mk(emit_pv, g, j, 0), 54 * (4 * j + 1)))
                for d in range(1, 4):
                    units.append((mk(emit_scores_exp, g, j, 4 * j + d),
                                  wsc(4 * j + d)))
                    units.append((mk(emit_pv, g, j, d),
                                  54 * (4 * j + d + 1)))
                    units.append((mk(emit_norm, g, j, d - 1), 0))
                    if d >= 2:
                        units.append((mk(emit_tr, g, j, d - 2), 90))
                units.append((mk(emit_norm, g, j, 3), 0))
                units.append((mk(emit_tr, g, j, 2), 90))
                carry = mk(emit_tr, g, j, 3)
                groups.append(units)
            return groups, carry

        def zip_emit(primary, filler):
            tot_p = sum(c for _, c in primary) or 1
            tot_f = sum(c for _, c in filler) or 1
            n_f = len(filler)
            fi = 0
            cum_p = 0
            cum_f = 0
            for u, c in primary:
                u()
                cum_p += c
                while fi < n_f and cum_f * tot_p <= cum_p * tot_f:
                    fu, fc = filler[fi]
                    fu()
                    cum_f += fc
                    fi += 1
            while fi < n_f:
                filler[fi][0]()
                fi += 1

        carry = None
        for g in range(3):
            groups, carry = att_unit_groups(g, carry)
            zip_emit([u for grp in groups for u in grp],
                     emit_proj_units(g + 1, pp_pool))

        w_cm.__exit__(None, None, None)
        xt_cm.__exit__(None, None, None)

        wo_pool = es.enter_context(tc.tile_pool(name="wo", bufs=4))
        os_pool = es.enter_context(tc.tile_pool(name="os", bufs=4))
        wo_t = []
        for fc in range(4):
            w_t = wo_pool.tile([128, D], BF16, name=f"wo{fc}", tag=f"wo{fc}")
            nc.sync.dma_start(
                out=w_t, in_=wo_d[fc * 128:(fc + 1) * 128, :])
            wo_t.append(w_t)

        def outproj_units(j, pool=None):
            units = []
            for mt in range(4 * j, 4 * j + 4):
                for nck in range(2):
                    st8 = {}

                    def mk_g(j=j, mt=mt, nck=nck, g=0, st8=st8):
                        def u():
                            msl = slice((mt - 4 * j) * 128,
                                        (mt - 4 * j) * 128 + 128)
                            if g == 0:
                                st8["op"] = (pool or pp_pool).tile(
                                    [128, 512], F32,
                                    name=f"op{mt}_{nck}", tag="pp")
                            nc.tensor.matmul(
                                st8["op"][:],
                                ot_map[(j, g)][:, msl],
                                wo_t[g][:, nck * 512:(nck + 1) * 512],
                                start=(g == 0), stop=(g == 3))
                            if g == 3:
                                op = st8.pop("op")
                                osb = os_pool.tile(
                                    [128, 512], BF16,
                                    name=f"os{mt}_{nck}", tag="os")
                                nc.vector.tensor_copy(osb[:], op[:])
                                nc.sync.dma_start(
                                    out=part_d[
                                        mt * 128:(mt + 1) * 128,
                                        nck * 512:(nck + 1) * 512],
                                    in_=osb[:])
                        return u

                    for g in range(4):
                        units.append((mk_g(g=g), 245))
            return units

        groups, carry = att_unit_groups(3, carry)
        for j in range(4):
            zip_emit(groups[j], outproj_units(j - 1) if j > 0 else [])
        carry()  # tr(3, 3)
        # Free the attention PSUM pools (LIFO) and give the tail
        # out-projection a deep chain pool so its 8 accumulation chains
        # pipeline instead of serializing on one bank.
        tp_cm.__exit__(None, None, None)
        pp_cm.__exit__(None, None, None)
        pv_cm.__exit__(None, None, None)
        st_cm.__exit__(None, None, None)
        with tc.tile_pool(name="opt", bufs=6, space="PSUM") as op_pool:
            for u, _ in outproj_units(3, pool=op_pool):
                u()


def _get_program():
    global _PROGRAM
    if _PROGRAM is None:
        _PROGRAM = _build_program()
    return _PROGRAM


_EXEC = None


def _get_executor():
    global _EXEC
    if _EXEC is None:
        import jax
        from jax.experimental.shard_map import shard_map
        from jax.sharding import Mesh, PartitionSpec

        from concourse import bass2jax

        bass2jax.install_neuronx_cc_hook()
        nc = _get_program()
        part_name = (nc.partition_id_tensor.name
                     if nc.partition_id_tensor else None)
        in_names, out_names, out_avals = [], [], []
        for alloc in nc.m.functions[0].allocations:
            if not isinstance(alloc, mybir.MemoryLocationSet):
                continue
            name = alloc.memorylocations[0].name
            if alloc.kind == "ExternalInput":
                if name != part_name:
                    in_names.append(name)
            elif alloc.kind == "ExternalOutput":
                out_names.append(name)
                out_avals.append(jax.core.ShapedArray(
                    tuple(alloc.tensor_shape), mybir.dt.np(alloc.dtype)))
        n_params = len(in_names)
        all_in = tuple(in_names) + tuple(out_names)
        if part_name is not None:
            all_in = all_in + (part_name,)

        def _body(*args):
            operands = list(args)
            if part_name is not None:
                operands.append(bass2jax.partition_id_tensor())
            outs = bass2jax._bass_exec_p.bind(
                *operands,
                out_avals=tuple(out_avals),
                in_names=all_in,
                out_names=tuple(out_names),
                lowering_input_output_aliases=(),
                sim_require_finite=True,
                sim_require_nnan=True,
                nc=nc)
            return tuple(outs)

        devices = jax.devices()[:N_CORES]
        mesh = Mesh(np.asarray(devices), ("core",))
        n_bufs = n_params + len(out_names)
        mapped = shard_map(_body, mesh=mesh,
                           in_specs=(PartitionSpec("core"),) * n_bufs,
                           out_specs=(PartitionSpec("core"),) * len(out_names),
                           check_rep=False)
        fn = jax.jit(mapped,
                     donate_argnums=tuple(range(n_params, n_bufs)),
                     keep_unused=True)
        fn_nodonate = jax.jit(mapped, keep_unused=True)
        out_shapes = [tuple(a.shape) for a in out_avals]
        out_dtypes = [a.dtype for a in out_avals]
        _EXEC = (fn, fn_nodonate, in_names, out_names, out_shapes, mesh,
                 out_dtypes)
    return _EXEC


def run_cores(in_maps):
    ex = _get_executor()
    fn, _, in_names, out_names, out_shapes = ex[:5]
    out_dtypes = ex[6]
    concat_in = [np.concatenate([in_maps[c][n] for c in range(N_CORES)],
                                axis=0) for n in in_names]
    zeros = [np.zeros((N_CORES * s[0],) + s[1:], dt)
             for s, dt in zip(out_shapes, out_dtypes)]
    outs = fn(*concat_in, *zeros)
    res = []
    for c in range(N_CORES):
        res.append({
            n: np.asarray(outs[i]).reshape((N_CORES,) + out_shapes[i])[c]
            for i, n in enumerate(out_names)})
    return res


def make_in_maps(x, w_q, b_q, w_k, b_k, w_v, b_v, w_o, b_o):
    in_maps = []
    for c in range(N_CORES):
        b, hh = divmod(c, 2)
        cols = slice(hh * DH, (hh + 1) * DH)
        in_maps.append({
            "x": np.ascontiguousarray(x[b].T).astype(NPBF16),
            "wq": np.ascontiguousarray(w_q[:, cols]).astype(NPBF16),
            "wk": np.ascontiguousarray(w_k[:, cols]).astype(NPBF16),
            "wv": np.ascontiguousarray(w_v[:, cols]).astype(NPBF16),
            "wo": np.ascontiguousarray(w_o[cols, :]).astype(NPBF16),
            "bq": np.ascontiguousarray(b_q[cols]),
        })
    return in_maps


def combine(parts, b_v, w_o, b_o):
    corr = (b_v @ w_o + b_o).astype(np.float32)
    out = np.empty((4, S, D), dtype=np.float32)
    for b in range(4):
        out[b] = (parts[2 * b].astype(np.float32)
                  + parts[2 * b + 1].astype(np.float32) + corr)
    return out


def kernel(x, w_q, b_q, w_k, b_k, w_v, b_v, w_o, b_o):
    x = np.asarray(x, dtype=np.float32)
    w_q = np.asarray(w_q, dtype=np.float32)
    b_q = np.asarray(b_q, dtype=np.float32)
    w_k = np.asarray(w_k, dtype=np.float32)
    b_k = np.asarray(b_k, dtype=np.float32)
    w_v = np.asarray(w_v, dtype=np.float32)
    b_v = np.asarray(b_v, dtype=np.float32)
    w_o = np.asarray(w_o, dtype=np.float32)
    b_o = np.asarray(b_o, dtype=np.float32)

    in_maps = make_in_maps(x, w_q, b_q, w_k, b_k, w_v, b_v, w_o, b_o)
    res = run_cores(in_maps)
    parts = [res[c]["part"] for c in range(N_CORES)]
    return combine(parts, b_v, w_o, b_o)
